# revision 2
# baseline (speedup 1.0000x reference)
"""Trainium2 Bass kernel for AMM (landmark/Nystrom-style) attention.

Problem (per batch element b of 8, one NeuronCore each):
    qkv  = x @ W_qkv                     (4096,512)@(512,1536)
    q,k,v = split(qkv); q /= sqrt(512)
    keys_lm = segment_mean(k, 16)        (256,512)
    vals_lm = segment_mean(v, 16)        (256,512)
    attn = softmax(q @ keys_lm^T)        (4096,256)
    out  = attn @ vals_lm                (4096,512)
    out  = out @ W_proj + b_proj
    return v + out

Algebraic restructuring used here (exact in real arithmetic):
  - segment_mean commutes with the linear projections:
        keys_lm = pool(x) @ W_k,  vals_lm = pool(x) @ W_v
    so the full k matmul is never computed.
  - attn @ vals_lm @ W_proj = attn @ (vals_lm @ W_proj)  (256x512 through
    W_proj instead of 4096x512).
  - b_proj is folded into VW via a rank-1 (K=1) matmul: since softmax rows
    sum to 1 after normalization, attn @ (VW + 1*b) = attn @ VW + b.
  - softmax normalization is applied after the value matmul:
        out = (E @ VWb) * (1 / (E @ 1)) with E = exp(logits).

Sharding: pure data-parallel over batch B=8 across the 8 cores; weights are
replicated. No collectives. Host pre-transposes x per core to x^T (channel
dim on partitions) and casts matmul operands to bf16 (PSUM accumulation is
fp32; output is fp32). bf16 keeps the TensorEngine at 1 cycle/row (fp32
matmul runs at 4 cycles/row) and rel. error stays ~1e-3.
"""

import sys
from contextlib import ExitStack

import numpy as np

sys.path.insert(0, "/opt/trn_rl_repo")

import concourse.bass as bass  # noqa: E402
import concourse.tile as tile  # noqa: E402
from concourse import bacc, mybir  # noqa: E402
from concourse.bass_utils import run_bass_kernel_spmd  # noqa: E402

import ml_dtypes  # noqa: E402

BF16 = mybir.dt.bfloat16
F32 = mybir.dt.float32
AF = mybir.ActivationFunctionType
ALU = mybir.AluOpType

B, N, DIM = 8, 4096, 512
L, SEG = 256, 16  # landmarks, segment size
CT = DIM // 128  # 4 channel partition tiles
MT = N // 512  # 8 m-chunks of 512
RSCALE = 1.0 / float(np.sqrt(512.0))


def build_kernel(ctx: ExitStack, tc: "tile.TileContext", out_d, xt_d, wqkv_d, wproj_d, bproj_d):
    nc = tc.nc

    consts = ctx.enter_context(tc.tile_pool(name="consts", bufs=1))
    work = ctx.enter_context(tc.tile_pool(name="work", bufs=3))
    psum = ctx.enter_context(tc.tile_pool(name="psum", bufs=5, space="PSUM"))
    psden = ctx.enter_context(tc.tile_pool(name="psden", bufs=2, space="PSUM"))

    # ---- weights / constants ------------------------------------------------
    wqkv = consts.tile([128, CT, 3 * DIM], BF16)
    for j in range(CT):
        nc.sync.dma_start(out=wqkv[:, j, :], in_=wqkv_d[j, :, :])
    wproj = consts.tile([128, CT, DIM], BF16)
    for j in range(CT):
        nc.sync.dma_start(out=wproj[:, j, :], in_=wproj_d[j, :, :])
    bproj = consts.tile([1, DIM], BF16)
    nc.sync.dma_start(out=bproj[:, :], in_=bproj_d[:, :])

    ones_col = consts.tile([128, 1], BF16)
    nc.vector.memset(ones_col[:, :], 1.0)
    ones_row = consts.tile([1, 128], BF16)
    nc.vector.memset(ones_row[:, :], 1.0)

    # ---- x^T and landmark pooling ------------------------------------------
    xt = consts.tile([128, CT, N], BF16)
    xpool = consts.tile([128, CT, L], BF16)  # pool(x)^T (true mean)
    for j in range(CT):
        for mi in range(MT):
            nc.sync.dma_start(
                out=xt[:, j, mi * 512 : (mi + 1) * 512],
                in_=xt_d[j, :, mi * 512 : (mi + 1) * 512],
            )
        pf = work.tile([128, L], F32, tag="poolf")
        nc.vector.reduce_sum(
            pf[:, :],
            xt[:, j, :].rearrange("p (l s) -> p l s", s=SEG),
            axis=mybir.AxisListType.X,
        )
        nc.vector.tensor_scalar_mul(xpool[:, j, :], pf[:, :], 1.0 / SEG)

    # ---- landmark projections: keys_lm^T, vals_lm^T [d_part, L] ------------
    keysT = consts.tile([128, CT, L], BF16)
    valsT = consts.tile([128, CT, L], BF16)
    for dst, col0 in ((keysT, DIM), (valsT, 2 * DIM)):
        for dj in range(CT):
            pt = psum.tile([128, L], F32, tag="mm")
            for cj in range(CT):
                nc.tensor.matmul(
                    pt[:, :],
                    wqkv[:, cj, col0 + dj * 128 : col0 + (dj + 1) * 128],
                    xpool[:, cj, :],
                    start=(cj == 0),
                    stop=(cj == CT - 1),
                )
            nc.vector.tensor_copy(dst[:, dj, :], pt[:, :])

    # ---- VWb = vals_lm @ W_proj + 1*b  [l_part, 512] ------------------------
    vw = consts.tile([128, 2, DIM], BF16)
    for li in range(2):
        pt = psum.tile([128, DIM], F32, tag="mm")
        for dj in range(CT):
            nc.tensor.matmul(
                pt[:, :],
                valsT[:, dj, li * 128 : (li + 1) * 128],
                wproj[:, dj, :],
                start=(dj == 0),
                stop=False,
            )
        nc.tensor.matmul(pt[:, :], ones_row[:, :], bproj[:, :], start=False, stop=True)
        nc.vector.tensor_copy(vw[:, li, :], pt[:, :])

    # ---- q^T = (x @ W_q)^T  [d_part, N] -------------------------------------
    qt = consts.tile([128, CT, N], BF16)
    for dj in range(CT):
        for mi in range(MT):
            pt = psum.tile([128, 512], F32, tag="mm")
            for cj in range(CT):
                nc.tensor.matmul(
                    pt[:, :],
                    wqkv[:, cj, dj * 128 : (dj + 1) * 128],
                    xt[:, cj, mi * 512 : (mi + 1) * 512],
                    start=(cj == 0),
                    stop=(cj == CT - 1),
                )
            nc.vector.tensor_copy(qt[:, dj, mi * 512 : (mi + 1) * 512], pt[:, :])

    # ---- attention + projection + residual, per 512-row chunk ---------------
    for mi in range(MT):
        # logits^T [l_part(2x128), 512]; E = exp(logits/sqrt(512)) in bf16
        et = work.tile([128, 2, 512], BF16, tag="et")
        for li in range(2):
            pt = psum.tile([128, 512], F32, tag="mm")
            for dj in range(CT):
                nc.tensor.matmul(
                    pt[:, :],
                    keysT[:, dj, li * 128 : (li + 1) * 128],
                    qt[:, dj, mi * 512 : (mi + 1) * 512],
                    start=(dj == 0),
                    stop=(dj == CT - 1),
                )
            nc.scalar.activation(et[:, li, :], pt[:, :], AF.Exp, scale=RSCALE)

        for t in range(4):
            r0 = mi * 512 + t * 128
            sl = slice(t * 128, (t + 1) * 128)
            # denominator: E^T-column sums via N=1 matmul
            dp = psden.tile([128, 1], F32, tag="den")
            for li in range(2):
                nc.tensor.matmul(
                    dp[:, :], et[:, li, sl], ones_col[:, :],
                    start=(li == 0), stop=(li == 1),
                )
            rr = work.tile([128, 1], F32, tag="rr")
            nc.vector.reciprocal(rr[:, :], dp[:, :])
            # v tile [128, 512] (for the residual)
            vp = psum.tile([128, 512], F32, tag="mm")
            for cj in range(CT):
                nc.tensor.matmul(
                    vp[:, :],
                    xt[:, cj, r0 : r0 + 128],
                    wqkv[:, cj, 2 * DIM : 3 * DIM],
                    start=(cj == 0),
                    stop=(cj == CT - 1),
                )
            # out2 = E @ VWb
            op = psum.tile([128, 512], F32, tag="mm")
            for li in range(2):
                nc.tensor.matmul(
                    op[:, :], et[:, li, sl], vw[:, li, :],
                    start=(li == 0), stop=(li == 1),
                )
            # final = out2 * (1/den) + v   (HW: max one PSUM input per DVE op,
            # so v goes through SBUF first)
            vs = work.tile([128, 512], F32, tag="vs")
            nc.vector.tensor_copy(vs[:, :], vp[:, :])
            fin = work.tile([128, 512], F32, tag="fin")
            nc.vector.scalar_tensor_tensor(
                fin[:, :], op[:, :], rr[:, :], vs[:, :],
                op0=ALU.mult, op1=ALU.add,
            )
            nc.sync.dma_start(out=out_d[r0 : r0 + 128, :], in_=fin[:, :])


def build_nc():
    nc = bacc.Bacc("TRN2", target_bir_lowering=False, debug=False, num_devices=8)
    xt_d = nc.declare_dram_parameter("xt", [CT, 128, N], BF16, isOutput=False)
    wqkv_d = nc.declare_dram_parameter("wqkv", [CT, 128, 3 * DIM], BF16, isOutput=False)
    wproj_d = nc.declare_dram_parameter("wproj", [CT, 128, DIM], BF16, isOutput=False)
    bproj_d = nc.declare_dram_parameter("bproj", [1, DIM], BF16, isOutput=False)
    out_d = nc.declare_dram_parameter("out", [N, DIM], F32, isOutput=True)
    with tile.TileContext(nc) as tc, ExitStack() as ctx:
        build_kernel(ctx, tc, out_d.ap(), xt_d.ap(), wqkv_d.ap(), wproj_d.ap(), bproj_d.ap())
    nc.compile()
    return nc


def prep_in_maps(x, W_qkv, W_proj, b_proj):
    bf = ml_dtypes.bfloat16
    wq = np.ascontiguousarray(np.asarray(W_qkv, np.float32).astype(bf).reshape(CT, 128, 3 * DIM))
    wp = np.ascontiguousarray(np.asarray(W_proj, np.float32).astype(bf).reshape(CT, 128, DIM))
    bp = np.asarray(b_proj, np.float32).astype(bf).reshape(1, DIM)
    in_maps = []
    for i in range(B):
        xti = np.ascontiguousarray(np.asarray(x[i], np.float32).T.astype(bf)).reshape(CT, 128, N)
        in_maps.append({"xt": xti, "wqkv": wq, "wproj": wp, "bproj": bp})
    return in_maps


_NC_CACHE = None


def kernel(x, W_qkv, W_proj, b_proj):
    global _NC_CACHE
    if _NC_CACHE is None:
        _NC_CACHE = build_nc()
    nc = _NC_CACHE
    in_maps = prep_in_maps(x, W_qkv, W_proj, b_proj)
    res = run_bass_kernel_spmd(nc, in_maps, core_ids=list(range(B)))
    out = np.stack([res.results[i]["out"] for i in range(B)], axis=0)
    return out.astype(np.float32)


# revision 8
# speedup vs baseline: 23629.5984x; 23629.5984x over previous
"""Trainium2 Bass kernel for AMM (landmark/Nystrom-style) attention.

Problem (per batch element b of 8, one NeuronCore each):
    qkv  = x @ W_qkv                     (4096,512)@(512,1536)
    q,k,v = split(qkv); q /= sqrt(512)
    keys_lm = segment_mean(k, 16)        (256,512)
    vals_lm = segment_mean(v, 16)        (256,512)
    attn = softmax(q @ keys_lm^T)        (4096,256)
    out  = attn @ vals_lm                (4096,512)
    out  = out @ W_proj + b_proj
    return v + out

Algebraic restructuring used here (exact in real arithmetic):
  - segment_mean commutes with the linear projections:
        keys_lm = pool(x) @ W_k,  vals_lm = pool(x) @ W_v
    so the full k matmul is never computed.
  - attn @ vals_lm @ W_proj = attn @ (vals_lm @ W_proj)  (256x512 through
    W_proj instead of 4096x512).
  - b_proj is folded into VW via a rank-1 (K=1) matmul: since softmax rows
    sum to 1 after normalization, attn @ (VW + 1*b) = attn @ VW + b.
  - softmax normalization is applied after the value matmul:
        out = (E @ VWb) * (1 / (E @ 1)) with E = exp(logits).

Sharding: pure data-parallel over batch B=8 across the 8 cores; weights are
replicated. No collectives. Host pre-transposes x per core to x^T (channel
dim on partitions) and casts matmul operands to bf16 (PSUM accumulation is
fp32; output is fp32). bf16 keeps the TensorEngine at 1 cycle/row (fp32
matmul runs at 4 cycles/row) and rel. error stays ~1e-3.
"""

import sys
from contextlib import ExitStack

import numpy as np

sys.path.insert(0, "/opt/trn_rl_repo")

import concourse.bass as bass  # noqa: E402
import concourse.tile as tile  # noqa: E402
from concourse import bacc, mybir  # noqa: E402
from concourse.bass_utils import run_bass_kernel_spmd  # noqa: E402

import ml_dtypes  # noqa: E402

BF16 = mybir.dt.bfloat16
F32 = mybir.dt.float32
AF = mybir.ActivationFunctionType
ALU = mybir.AluOpType

B, N, DIM = 8, 4096, 512
L, SEG = 256, 16  # landmarks, segment size
CT = DIM // 128  # 4 channel partition tiles
MT = N // 512  # 8 m-chunks of 512
RSCALE = 1.0 / float(np.sqrt(512.0))


def build_kernel(ctx: ExitStack, tc: "tile.TileContext", out_d, xt_d, wqkv_d, wproj_d, bproj_d):
    nc = tc.nc

    consts = ctx.enter_context(tc.tile_pool(name="consts", bufs=1))
    work = ctx.enter_context(tc.tile_pool(name="work", bufs=3))
    vpool = ctx.enter_context(tc.tile_pool(name="vpool", bufs=6))
    psum = ctx.enter_context(tc.tile_pool(name="psum", bufs=4, space="PSUM"))
    psumv = ctx.enter_context(tc.tile_pool(name="psumv", bufs=2, space="PSUM"))
    psden = ctx.enter_context(tc.tile_pool(name="psden", bufs=2, space="PSUM"))

    # ---- weights / constants ------------------------------------------------
    wqkv = consts.tile([128, CT, 3 * DIM], BF16)
    for j in range(CT):
        nc.sync.dma_start(out=wqkv[:, j, :], in_=wqkv_d[j, :, :])

    ones_col = consts.tile([128, 1], BF16)
    nc.vector.memset(ones_col[:, :], 1.0)
    ones_row = consts.tile([1, 128], BF16)
    nc.vector.memset(ones_row[:, :], 1.0)

    # ---- x^T (m-chunk-major so qT can start early) + partial pooling -------
    xt = consts.tile([128, CT, N], BF16)
    poolf = consts.tile([128, CT, L], F32)  # segment sums (f32)
    xpool = consts.tile([128, CT, L], BF16)  # pool(x)^T (true mean)
    LC = L // MT  # landmarks covered per 512-wide chunk (32)
    for mi in range(MT):
        for j in range(CT):
            nc.sync.dma_start(
                out=xt[:, j, mi * 512 : (mi + 1) * 512],
                in_=xt_d[j, :, mi * 512 : (mi + 1) * 512],
            )
        for j in range(CT):
            nc.vector.reduce_sum(
                poolf[:, j, mi * LC : (mi + 1) * LC],
                xt[:, j, mi * 512 : (mi + 1) * 512].rearrange(
                    "p (l s) -> p l s", s=SEG
                ),
                axis=mybir.AxisListType.X,
            )
    for j in range(CT):
        nc.vector.tensor_scalar_mul(xpool[:, j, :], poolf[:, j, :], 1.0 / SEG)

    wproj = consts.tile([128, CT, DIM], BF16)
    for j in range(CT):
        nc.sync.dma_start(out=wproj[:, j, :], in_=wproj_d[j, :, :])
    bproj = consts.tile([1, DIM], BF16)
    nc.sync.dma_start(out=bproj[:, :], in_=bproj_d[:, :])

    # ---- q^T = (x @ W_q)^T  [d_part, N] -------------------------------------
    # (issued first in program order: only needs wqkv + the mi-th x chunk, so
    # the PE starts here while the pooling/landmark chain is still waiting on
    # the x DMA tail; psum->sbuf copies go to the otherwise-idle ACT engine)
    qt = consts.tile([128, CT, N], BF16)
    for mi in range(MT):
        for dj in range(CT):
            pt = psum.tile([128, 512], F32, tag="mm")
            for cj in range(CT):
                nc.tensor.matmul(
                    pt[:, :],
                    wqkv[:, cj, dj * 128 : (dj + 1) * 128],
                    xt[:, cj, mi * 512 : (mi + 1) * 512],
                    start=(cj == 0),
                    stop=(cj == CT - 1),
                )
            nc.scalar.copy(qt[:, dj, mi * 512 : (mi + 1) * 512], pt[:, :])

    # ---- landmark projections: keys_lm^T, vals_lm^T [d_part, L] ------------
    keysT = consts.tile([128, CT, L], BF16)
    valsT = consts.tile([128, CT, L], BF16)
    for dst, col0 in ((keysT, DIM), (valsT, 2 * DIM)):
        for dj in range(CT):
            pt = psum.tile([128, L], F32, tag="mm")
            for cj in range(CT):
                nc.tensor.matmul(
                    pt[:, :],
                    wqkv[:, cj, col0 + dj * 128 : col0 + (dj + 1) * 128],
                    xpool[:, cj, :],
                    start=(cj == 0),
                    stop=(cj == CT - 1),
                )
            nc.scalar.copy(dst[:, dj, :], pt[:, :])

    # ---- VWb = vals_lm @ W_proj + 1*b  [l_part, 512] ------------------------
    vw = consts.tile([128, 2, DIM], BF16)
    for li in range(2):
        pt = psum.tile([128, DIM], F32, tag="mm")
        for dj in range(CT):
            nc.tensor.matmul(
                pt[:, :],
                valsT[:, dj, li * 128 : (li + 1) * 128],
                wproj[:, dj, :],
                start=(dj == 0),
                stop=False,
            )
        nc.tensor.matmul(pt[:, :], ones_row[:, :], bproj[:, :], start=False, stop=True)
        nc.scalar.copy(vw[:, li, :], pt[:, :])

    # ---- attention + projection + residual, per 512-row chunk ---------------
    for mi in range(MT):
        # logits^T [l_part(2x128), 512]; E = exp(logits/sqrt(512)) in bf16
        et = work.tile([128, 2, 512], BF16, tag="et")
        for li in range(2):
            pt = psum.tile([128, 512], F32, tag="mm")
            for dj in range(CT):
                nc.tensor.matmul(
                    pt[:, :],
                    keysT[:, dj, li * 128 : (li + 1) * 128],
                    qt[:, dj, mi * 512 : (mi + 1) * 512],
                    start=(dj == 0),
                    stop=(dj == CT - 1),
                )
            nc.scalar.activation(et[:, li, :], pt[:, :], AF.Exp, scale=RSCALE)

        # v tiles first: independent of exp, keeps the PE dense while ACT exps
        vps = []
        for t in range(4):
            r0 = mi * 512 + t * 128
            vp = psumv.tile([128, 512], F32, tag="mmv")
            for cj in range(CT):
                nc.tensor.matmul(
                    vp[:, :],
                    xt[:, cj, r0 : r0 + 128],
                    wqkv[:, cj, 2 * DIM : 3 * DIM],
                    start=(cj == 0),
                    stop=(cj == CT - 1),
                )
            vs = vpool.tile([128, 512], F32, tag="vs")
            nc.vector.tensor_copy(vs[:, :], vp[:, :])
            vps.append(vs)

        for t in range(4):
            r0 = mi * 512 + t * 128
            sl = slice(t * 128, (t + 1) * 128)
            # denominator: E^T-column sums via N=1 matmul
            dp = psden.tile([128, 1], F32, tag="den")
            for li in range(2):
                nc.tensor.matmul(
                    dp[:, :], et[:, li, sl], ones_col[:, :],
                    start=(li == 0), stop=(li == 1),
                )
            rr = work.tile([128, 1], F32, tag="rr")
            nc.vector.reciprocal(rr[:, :], dp[:, :])
            # out2 = E @ VWb
            op = psum.tile([128, 512], F32, tag="mm")
            for li in range(2):
                nc.tensor.matmul(
                    op[:, :], et[:, li, sl], vw[:, li, :],
                    start=(li == 0), stop=(li == 1),
                )
            # final = out2 * (1/den) + v   (HW: max one PSUM input per DVE op)
            fin = work.tile([128, 512], F32, tag="fin")
            nc.vector.scalar_tensor_tensor(
                fin[:, :], op[:, :], rr[:, :], vps[t][:, :],
                op0=ALU.mult, op1=ALU.add,
            )
            nc.sync.dma_start(out=out_d[r0 : r0 + 128, :], in_=fin[:, :])


def build_nc(repeat: int = 1):
    nc = bacc.Bacc("TRN2", target_bir_lowering=False, debug=False, num_devices=8)
    xt_d = nc.declare_dram_parameter("xt", [CT, 128, N], BF16, isOutput=False)
    wqkv_d = nc.declare_dram_parameter("wqkv", [CT, 128, 3 * DIM], BF16, isOutput=False)
    wproj_d = nc.declare_dram_parameter("wproj", [CT, 128, DIM], BF16, isOutput=False)
    bproj_d = nc.declare_dram_parameter("bproj", [1, DIM], BF16, isOutput=False)
    out_d = nc.declare_dram_parameter("out", [N, DIM], F32, isOutput=True)
    with tile.TileContext(nc) as tc, ExitStack() as ctx:
        if repeat == 1:
            build_kernel(ctx, tc, out_d.ap(), xt_d.ap(), wqkv_d.ap(), wproj_d.ap(), bproj_d.ap())
        else:
            # benchmarking mode: run the whole body `repeat` times in a HW loop
            with tc.For_i(0, repeat, 1):
                build_kernel(ctx, tc, out_d.ap(), xt_d.ap(), wqkv_d.ap(), wproj_d.ap(), bproj_d.ap())
    nc.compile()
    return nc


def prep_in_maps(x, W_qkv, W_proj, b_proj):
    bf = ml_dtypes.bfloat16
    wq = np.ascontiguousarray(np.asarray(W_qkv, np.float32).astype(bf).reshape(CT, 128, 3 * DIM))
    wp = np.ascontiguousarray(np.asarray(W_proj, np.float32).astype(bf).reshape(CT, 128, DIM))
    bp = np.asarray(b_proj, np.float32).astype(bf).reshape(1, DIM)
    in_maps = []
    for i in range(B):
        xti = np.ascontiguousarray(np.asarray(x[i], np.float32).T.astype(bf)).reshape(CT, 128, N)
        in_maps.append({"xt": xti, "wqkv": wq, "wproj": wp, "bproj": bp})
    return in_maps


_NC_CACHE = None


def kernel(x, W_qkv, W_proj, b_proj):
    global _NC_CACHE
    if _NC_CACHE is None:
        _NC_CACHE = build_nc()
    nc = _NC_CACHE
    in_maps = prep_in_maps(x, W_qkv, W_proj, b_proj)
    res = run_bass_kernel_spmd(nc, in_maps, core_ids=list(range(B)))
    out = np.stack([res.results[i]["out"] for i in range(B)], axis=0)
    return out.astype(np.float32)
